# revision 4
# baseline (speedup 1.0000x reference)
"""Trainium2 Bass kernel for the Actor net (patch relabel + MLP), 8-core SPMD.

Strategy: data-parallel over the B*7396 patch-row axis. Host extracts the
3x3 non-overlapping patches (offset decoded from x[0,0,0,0]) into a
feature-major tensor, sharded by rows across 8 cores. Single fused
software-pipelined loop (keeps the PE HAM clock-gate warm at 2.4 GHz).

Relabel (per 2048-col super, 4 tiles strip-packed at 32-row offsets):
  - Host ships pair-space replications of the 9 ch0 values: A[q=(u,v)]=a_v,
    B[q]=a_u (81 rows each). Device: DIF = A + (-B) (DVE negate + accum-DMA),
    EQ9 = 9*[DIF==0] (ts, 4x mode).
  - S (dup-count before u) via one 4-way col-tiled matmul group; fo=relu(1-S)
    on Act; d (cumulative first-occurrence count) via 4-way diag-tiled
    matmuls; fd = fo*d; fdm = fd*[a_v!=0] (the output mask folds to the
    v-side because eq forces a_u==a_v) -- all in cheap strip space.
  - fdm strips -> FDr rows 0:9 -> log-doubling DMA replication to 81 rows;
    FDr += EQ9 (accum-DMA); PR' = relu(FDr - 9) = eq*fd*mask (ts, 4x).
  - The uni contraction (sum over v) is folded into the first MLP layer:
    W1B rows 0:81 replicate W1's ch0 rows, so
    L1 = W1A^T @ rest128 + W1B^T @ [PR'; rest7]  (K=128 + K=88).
MLP: L2 256x256, L3+heads folded into one [256,8] matrix (host folds W3 and
the two heads; output biases added on the host).
"""
import sys

sys.path.insert(0, "/opt/trn_rl_repo")

import numpy as np

H = W = 256
PATCH = 3
PH = 86
C = 16
B = 32
NCORES = 8
M_TOTAL = B * PH * PH            # 236672 patch rows
NC_CORE = M_TOTAL // NCORES      # 29584 rows per core
TILE_N = 512
TPS = 4                          # tiles per super (strip-packed)
SUP = TILE_N * TPS               # 2048
NSUP = 15
NCP = NSUP * SUP                 # 30720 padded columns per core
NT = NCP // TILE_N               # 60
D_IN = PATCH * PATCH * C         # 144

TRACE = False
TRACE_KWARGS = {}
LAST_EXEC_NS = None
LAST_RESULT = None

# feature index maps into the raw (M, 144) patch matrix
_RESTIDX = [p * C + c for p in range(9) for c in range(1, C)]   # 135 rows
_CH0IDX = [p * C for p in range(9)]                              # 9 rows
_IDXA = np.tile(np.arange(9), 9)      # q=(u,v) -> v
_IDXB = np.repeat(np.arange(9), 9)    # q=(u,v) -> u


def _relabel_consts():
    TLT = np.zeros((81, 9), np.float32)   # S_u = sum_{v<u} eq[(u,v)]
    TINC = np.zeros((9, 9), np.float32)   # d_m = sum_{k<=m} fo_k
    for u in range(9):
        for v in range(9):
            if v < u:
                TLT[u * 9 + v, u] = 1.0
    for k in range(9):
        for m in range(9):
            if k <= m:
                TINC[k, m] = 1.0
    return TLT, TINC


_GRAPH = None


def _build_graph():
    global _GRAPH
    if _GRAPH is not None:
        return _GRAPH
    import concourse.bass as bass
    import concourse.bacc as bacc
    import concourse.mybir as mybir
    import concourse.tile as tile

    bf16 = mybir.dt.bfloat16
    f32 = mybir.dt.float32
    AF = mybir.ActivationFunctionType
    ALU = mybir.AluOpType

    nc = bacc.Bacc("TRN2")
    feat_e = nc.declare_dram_parameter("feat", [144, NCP], bf16, isOutput=False)
    rep_e = nc.declare_dram_parameter("rep", [162, NCP], bf16, isOutput=False)
    w1a_e = nc.declare_dram_parameter("w1a", [128, 256], bf16, isOutput=False)
    w1b_e = nc.declare_dram_parameter("w1b", [88, 256], bf16, isOutput=False)
    w2_e = nc.declare_dram_parameter("w2", [256, 256], bf16, isOutput=False)
    whx_e = nc.declare_dram_parameter("whx", [256, 8], bf16, isOutput=False)
    b1_e = nc.declare_dram_parameter("b1", [256, 1], f32, isOutput=False)
    b2_e = nc.declare_dram_parameter("b2", [256, 1], f32, isOutput=False)
    tlt_e = nc.declare_dram_parameter("tlt", [81, 9], bf16, isOutput=False)
    tinc_e = nc.declare_dram_parameter("tinc", [9, 9], bf16, isOutput=False)
    bfo_e = nc.declare_dram_parameter("bfo", [128, 1], f32, isOutput=False)
    out_e = nc.declare_dram_parameter("out", [128, NSUP * TILE_N], f32,
                                      isOutput=True)

    def scol(s):
        return bass.ts(s, SUP)

    def tcol(t):
        return bass.ts(t, TILE_N)

    with tile.TileContext(nc) as tc:
        with (
            tc.tile_pool(name="const", bufs=1) as cp,
            tc.tile_pool(name="sb", bufs=3) as sb,
            tc.tile_pool(name="ps", bufs=2, space=bass.MemorySpace.PSUM) as ps,
        ):
            def const_tile(src, shape, dtype, tag, eng=None):
                t = cp.tile(shape, dtype, tag=tag, name=tag)
                (eng or nc.sync).dma_start(t[:], src)
                return t

            gq = nc.gpsimd
            w1a0 = const_tile(w1a_e[:, 0:128], [128, 128], bf16, "w1a0", gq)
            w1a1 = const_tile(w1a_e[:, 128:256], [128, 128], bf16, "w1a1", gq)
            w1b0 = const_tile(w1b_e[:, 0:128], [88, 128], bf16, "w1b0", gq)
            w1b1 = const_tile(w1b_e[:, 128:256], [88, 128], bf16, "w1b1", gq)
            w2_00 = const_tile(w2_e[0:128, 0:128], [128, 128], bf16, "w2_00", gq)
            w2_01 = const_tile(w2_e[0:128, 128:256], [128, 128], bf16, "w2_01", gq)
            w2_10 = const_tile(w2_e[128:256, 0:128], [128, 128], bf16, "w2_10", gq)
            w2_11 = const_tile(w2_e[128:256, 128:256], [128, 128], bf16, "w2_11", gq)
            whx0 = const_tile(whx_e[0:128, :], [128, 8], bf16, "whx0", gq)
            whx1 = const_tile(whx_e[128:256, :], [128, 8], bf16, "whx1", gq)
            b1a = const_tile(b1_e[0:128, :], [128, 1], f32, "b1a", nc.scalar)
            b1b = const_tile(b1_e[128:256, :], [128, 1], f32, "b1b", nc.scalar)
            b2a = const_tile(b2_e[0:128, :], [128, 1], f32, "b2a", nc.scalar)
            b2b = const_tile(b2_e[128:256, :], [128, 1], f32, "b2b", nc.scalar)
            tlt = const_tile(tlt_e[:], [81, 9], bf16, "tlt", nc.sync)
            bfo = const_tile(bfo_e[:], [128, 1], f32, "bfo", nc.scalar)
            tincS = cp.tile([128, 9], bf16, tag="tincS", name="tincS")
            for t in range(TPS):
                nc.scalar.dma_start(tincS[32 * t:32 * t + 9, :], tinc_e[:])

            def in_rng(s):
                return 0 <= s < NSUP

            fa = {}; xb = {}; Br = {}; Brn = {}; DIF = {}; EQ9 = {}
            ch0S = {}; fo4 = {}; fd4 = {}; fdm = {}; FDr = {}
            psSD = {}; psH = {}; otS = {}
            h1a = {}; h1b = {}; h2a = {}; h2b = {}
            ps1a = {}; ps1b = {}; ps2a = {}; ps2b = {}

            LAG = 3
            for k in range(NSUP + LAG):
                s = k
                if in_rng(s):  # LD + DIF prep
                    fa[s] = sb.tile([128, SUP], bf16, tag="fa", bufs=4,
                                    name=f"fa_{s}")
                    nc.sync.dma_start(fa[s][:], feat_e[0:128, scol(s)])
                    xb[s] = sb.tile([88, SUP], bf16, tag="xb", bufs=4,
                                    name=f"xb_{s}")
                    nc.sync.dma_start(xb[s][81:88, :], feat_e[128:135, scol(s)])
                    ch0S[s] = sb.tile([128, TILE_N], bf16, tag="ch0S", bufs=2,
                                      name=f"ch0S_{s}")
                    for t in range(TPS):
                        nc.sync.dma_start(
                            ch0S[s][32 * t:32 * t + 9, :],
                            feat_e[135:144, tcol(TPS * s + t)])
                    Br[s] = sb.tile([81, SUP], bf16, tag="Br", bufs=2,
                                    name=f"Br_{s}")
                    nc.sync.dma_start(Br[s][:], rep_e[81:162, scol(s)])
                    DIF[s] = sb.tile([81, SUP], bf16, tag="DIF", bufs=2,
                                     name=f"DIF_{s}")
                    nc.sync.dma_start(DIF[s][:], rep_e[0:81, scol(s)])
                    Brn[s] = sb.tile([81, SUP], bf16, tag="Brn", bufs=2,
                                     name=f"Brn_{s}")
                    nc.vector.tensor_scalar(Brn[s][:], Br[s][:], -1.0, None,
                                            op0=ALU.mult)
                    del Br[s]
                    gq.dma_start(DIF[s][:], Brn[s][:], accum_op=ALU.add)
                    del Brn[s]

                s = k - 1
                if in_rng(s):  # relabel strip chain
                    EQ9[s] = sb.tile([81, SUP], bf16, tag="EQ9", bufs=2,
                                     name=f"EQ9_{s}")
                    nc.vector.tensor_scalar(EQ9[s][:], DIF[s][:], 0.0, 9.0,
                                            op0=ALU.is_equal, op1=ALU.mult)
                    del DIF[s]
                    psSD[s] = ps.tile([128, TILE_N], f32, tag="sd", bufs=1,
                                      name=f"psSD_{s}")
                    for t in range(TPS):
                        nc.tensor.matmul(psSD[s][32 * t:32 * t + 9, :], tlt[:],
                                         EQ9[s][:, tcol(t)], start=True,
                                         stop=True, tile_position=(0, 32 * t))
                    # S' = 9*S (from EQ9); relu(1 - S'/8) is exact: 1 at S=0,
                    # <=0 for S>=1 (9/8 > 1), and /8 is a power-of-2 scale.
                    fo4[s] = sb.tile([128, TILE_N], bf16, tag="fo4", bufs=2,
                                     name=f"fo4_{s}")
                    nc.scalar.activation(fo4[s][:], psSD[s][:], AF.Relu,
                                         bias=bfo[:], scale=-0.125)
                    for t in range(TPS):
                        nc.tensor.matmul(psSD[s][32 * t:32 * t + 9, :],
                                         tincS[32 * t:32 * t + 9, :],
                                         fo4[s][32 * t:32 * t + 9, :],
                                         start=True, stop=True,
                                         tile_position=(32 * t, 32 * t))
                    fd4[s] = sb.tile([128, TILE_N], bf16, tag="fd4", bufs=2,
                                     name=f"fd4_{s}")
                    nc.vector.scalar_tensor_tensor(
                        fd4[s][:], fo4[s][:], 0.0, psSD[s][:],
                        op0=ALU.bypass, op1=ALU.mult)
                    del psSD[s]; del fo4[s]
                    fdm[s] = sb.tile([128, TILE_N], bf16, tag="fdm", bufs=2,
                                     name=f"fdm_{s}")
                    nc.vector.scalar_tensor_tensor(
                        fdm[s][:], ch0S[s][:], 0.0, fd4[s][:],
                        op0=ALU.not_equal, op1=ALU.mult)
                    del fd4[s]; del ch0S[s]
                    FDr[s] = sb.tile([81, SUP], bf16, tag="FDr", bufs=2,
                                     name=f"FDr_{s}")
                    for t in range(TPS):
                        gq.dma_start(FDr[s][0:9, tcol(t)],
                                     fdm[s][32 * t:32 * t + 9, :])
                    gq.dma_start(FDr[s][9:18, :], FDr[s][0:9, :])
                    gq.dma_start(FDr[s][18:36, :], FDr[s][0:18, :])
                    gq.dma_start(FDr[s][36:72, :], FDr[s][0:36, :])
                    gq.dma_start(FDr[s][72:81, :], FDr[s][0:9, :])
                    del fdm[s]

                s = k - 2
                if in_rng(s):  # PR' = relu(FDr + EQ9 - 9) -> xb rows 0:81
                    gq.dma_start(FDr[s][:], EQ9[s][:], accum_op=ALU.add)
                    del EQ9[s]
                    nc.vector.tensor_scalar(xb[s][0:81, :], FDr[s][:],
                                            -9.0, 0.0,
                                            op0=ALU.add, op1=ALU.max)
                    del FDr[s]

                s = k - LAG
                if in_rng(s):  # MLP for the 4 tiles of super s
                    psH[s] = ps.tile([128, TILE_N], f32, tag="psH", bufs=1,
                                     name=f"psH_{s}")
                    for tp in range(TPS // 2):  # weight-grouped tile pairs
                        tt2 = (2 * tp, 2 * tp + 1)
                        for t in tt2:
                            ps1a[t] = ps.tile([128, TILE_N], f32, tag="ps1",
                                              bufs=4, name=f"ps1a_{s}_{t}")
                            ps1b[t] = ps.tile([128, TILE_N], f32, tag="ps1",
                                              bufs=4, name=f"ps1b_{s}_{t}")
                        for t in tt2:
                            nc.tensor.matmul(ps1a[t][:], w1a0[:],
                                             fa[s][:, tcol(t)],
                                             start=True, stop=False)
                        for t in tt2:
                            nc.tensor.matmul(ps1a[t][:], w1b0[:],
                                             xb[s][:, tcol(t)],
                                             start=False, stop=True)
                        for t in tt2:
                            nc.tensor.matmul(ps1b[t][:], w1a1[:],
                                             fa[s][:, tcol(t)],
                                             start=True, stop=False)
                        for t in tt2:
                            nc.tensor.matmul(ps1b[t][:], w1b1[:],
                                             xb[s][:, tcol(t)],
                                             start=False, stop=True)
                        for t in tt2:
                            h1a[t] = sb.tile([128, TILE_N], bf16, tag="h1a",
                                             bufs=4, name=f"h1a_{s}_{t}")
                            h1b[t] = sb.tile([128, TILE_N], bf16, tag="h1b",
                                             bufs=4, name=f"h1b_{s}_{t}")
                            nc.scalar.activation(h1a[t][:], ps1a[t][:],
                                                 AF.Relu, bias=b1a[:])
                            nc.scalar.activation(h1b[t][:], ps1b[t][:],
                                                 AF.Relu, bias=b1b[:])
                            del ps1a[t]; del ps1b[t]
                        for t in tt2:
                            ps2a[t] = ps.tile([128, TILE_N], f32, tag="ps2",
                                              bufs=2, name=f"ps2a_{s}_{t}")
                            nc.tensor.matmul(ps2a[t][:], w2_00[:], h1a[t][:],
                                             start=True, stop=False)
                            nc.tensor.matmul(ps2a[t][:], w2_10[:], h1b[t][:],
                                             start=False, stop=True)
                            ps2b[t] = ps.tile([128, TILE_N], f32, tag="ps2",
                                              bufs=2, name=f"ps2b_{s}_{t}")
                            nc.tensor.matmul(ps2b[t][:], w2_01[:], h1a[t][:],
                                             start=True, stop=False)
                            nc.tensor.matmul(ps2b[t][:], w2_11[:], h1b[t][:],
                                             start=False, stop=True)
                            h2a[t] = sb.tile([128, TILE_N], bf16, tag="h2a",
                                             bufs=3, name=f"h2a_{s}_{t}")
                            h2b[t] = sb.tile([128, TILE_N], bf16, tag="h2b",
                                             bufs=3, name=f"h2b_{s}_{t}")
                            if t == 0:
                                nc.scalar.activation(h2a[t][:], ps2a[t][:],
                                                     AF.Relu, bias=b2a[:])
                            else:
                                nc.vector.tensor_scalar(
                                    h2a[t][:], ps2a[t][:], b2a[:], 0.0,
                                    op0=ALU.add, op1=ALU.max)
                            nc.vector.tensor_scalar(
                                h2b[t][:], ps2b[t][:], b2b[:], 0.0,
                                op0=ALU.add, op1=ALU.max)
                            del ps2a[t]; del ps2b[t]
                            del h1a[t]; del h1b[t]
                    del fa[s]; del xb[s]
                    # L3 + heads: 8 matmuls col-group packed into one bank
                    for t in range(TPS):
                        nc.tensor.matmul(psH[s][32 * t:32 * t + 8, :], whx0[:],
                                         h2a[t][:], start=True, stop=False,
                                         tile_position=(0, 32 * t))
                    for t in range(TPS):
                        nc.tensor.matmul(psH[s][32 * t:32 * t + 8, :], whx1[:],
                                         h2b[t][:], start=False, stop=True,
                                         tile_position=(0, 32 * t))
                    for t in range(TPS):
                        del h2a[t]; del h2b[t]
                    otS[s] = sb.tile([128, TILE_N], f32, tag="otS", bufs=2,
                                     name=f"otS_{s}")
                    nc.scalar.activation(otS[s][:], psH[s][:], AF.Copy)
                    del psH[s]
                    nc.sync.dma_start(out_e[:, bass.ts(s, TILE_N)], otS[s][:])
                    del otS[s]

    nc.finalize()
    _GRAPH = nc
    return nc


def _extract_features(x):
    """numpy port of the reference's offset decode + patch extraction."""
    x = np.array(x, dtype=np.float32, copy=True)
    code = x[0, 0, 0, 0]
    it = np.float32(np.mod(code, np.float32(100.0)))
    x[0, 0, 0, 0] = np.float32((code - it) / np.float32(100.0))
    it_i = np.int32(it)
    off_h = int(it_i % 3)
    off_w = int((it_i // 3) % 3)
    xp = np.zeros((B, H + 4, W + 4, C), np.float32)
    xp[:, 2:2 + H, 2:2 + W, :] = x
    xp = xp[:, 2 - off_h:2 - off_h + H + 2, 2 - off_w:2 - off_w + W + 2, :]
    patches = xp.reshape(B, PH, PATCH, PH, PATCH, C)
    patches = patches.transpose(0, 1, 3, 2, 4, 5).reshape(M_TOTAL, PATCH * PATCH, C)
    return patches.reshape(M_TOTAL, D_IN)


_BFO = np.ones((128, 1), np.float32)


def kernel(x, W1, b1, W2, b2, W3, b3, Wm, bm, Wl, bl):
    global LAST_EXEC_NS, LAST_RESULT
    from concourse.bass_utils import run_bass_kernel_spmd
    import concourse.mybir as mybir

    bf16 = mybir.dt.np(mybir.dt.bfloat16)
    feat = _extract_features(np.asarray(x))

    TLT, TINC = _relabel_consts()
    W1 = np.asarray(W1, np.float32)
    W3 = np.asarray(W3, np.float32)
    b3 = np.asarray(b3, np.float32)
    Wm = np.asarray(Wm, np.float32)
    Wl = np.asarray(Wl, np.float32)
    whx = W3 @ np.concatenate([Wm, Wl], axis=1)          # [256, 8]
    bias8 = np.concatenate([b3 @ Wm + np.asarray(bm, np.float32),
                            b3 @ Wl + np.asarray(bl, np.float32)])  # [8]
    W1rest = W1[_RESTIDX, :]                              # [135, 256]
    W1ch0 = W1[_CH0IDX, :]                                # [9, 256]
    w1a = W1rest[0:128]
    w1b = np.concatenate([W1ch0[_IDXB], W1rest[128:135]], axis=0)  # [88, 256]
    common = dict(
        w1a=w1a.astype(bf16), w1b=np.ascontiguousarray(w1b).astype(bf16),
        w2=np.asarray(W2, np.float32).astype(bf16),
        whx=whx.astype(bf16),
        b1=np.asarray(b1, np.float32).reshape(256, 1),
        b2=np.asarray(b2, np.float32).reshape(256, 1),
        tlt=TLT.astype(bf16), tinc=TINC.astype(bf16),
        bfo=_BFO,
    )
    restT = feat[:, _RESTIDX].astype(bf16)                # [M, 135]
    ch0 = feat[:, _CH0IDX].astype(bf16)                   # [M, 9]
    in_maps = []
    for c in range(NCORES):
        lo, hi = c * NC_CORE, (c + 1) * NC_CORE
        ft = np.zeros((144, NCP), bf16)
        ft[0:135, :NC_CORE] = restT[lo:hi].T
        ft[135:144, :NC_CORE] = ch0[lo:hi].T
        rp = np.zeros((162, NCP), bf16)
        rp[0:81, :NC_CORE] = ch0[lo:hi][:, _IDXA].T
        rp[81:162, :NC_CORE] = ch0[lo:hi][:, _IDXB].T
        in_maps.append(dict(feat=np.ascontiguousarray(ft),
                            rep=np.ascontiguousarray(rp), **common))

    nc = _build_graph()
    res = run_bass_kernel_spmd(
        nc, in_maps, list(range(NCORES)), trace=TRACE, trace_kwargs=TRACE_KWARGS)
    LAST_EXEC_NS = res.exec_time_ns
    LAST_RESULT = res
    means, logs = [], []
    for c in range(NCORES):
        raw = res.results[c]["out"]                       # [128, NSUP*512]
        # row 32t+h, col 512s+cc  ->  head h of tile (4s+t)
        o = raw.reshape(4, 32, NSUP, TILE_N)[:, 0:8]      # [t, h, s, cc]
        o = o.transpose(1, 2, 0, 3).reshape(8, NCP)[:, :NC_CORE]
        o = o + bias8[:, None]
        means.append(o[0:4].T.reshape(B // NCORES, PH * PH * 4))
        logs.append(o[4:8].T.reshape(B // NCORES, PH * PH * 4))
    mean = np.concatenate(means, axis=0)
    log_std = np.concatenate(logs, axis=0)
    return mean, log_std


# revision 18
# speedup vs baseline: 1.4869x; 1.4869x over previous
"""Trainium2 Bass kernel for the Actor net (patch relabel + MLP), 8-core SPMD.

Strategy: data-parallel over the B*7396 patch-row axis. Host extracts the
3x3 non-overlapping patches (offset decoded from x[0,0,0,0]) into a
feature-major tensor, sharded by rows across 8 cores. Single fused
software-pipelined loop (keeps the PE HAM clock-gate warm at 2.4 GHz).

Relabel (per 2048-col super, 4 tiles strip-packed at 32-row offsets):
  - Host ships pair-space replications of the 9 ch0 values: A[q=(u,v)]=a_v,
    B[q]=a_u (81 rows each). Device: EQ = (A==B) on DVE.
  - S (dup-count before u) via one 4-way col-tiled matmul group; fo=relu(1-S)
    on Act; d (cumulative first-occurrence count) via 4-way diag-tiled
    matmuls; fd = fo*d on DVE.
  - fd strips are replicated to 81 pair rows by a log-doubling DMA chain.
  - PR' = (B!=0) * EQ * FDrep; the uni contraction (TALL) is folded into
    the first MLP layer: W1B rows 0:81 replicate W1's ch0 rows, so
    L1 = W1A^T @ rest128 + W1B^T @ [PR'; rest7]  (K=128 + K=88).
MLP: L2 256x256, L3+heads folded into one [256,8] matrix (host folds W3 and
the two heads; output biases added on the host).
"""
import sys

sys.path.insert(0, "/opt/trn_rl_repo")

import numpy as np

H = W = 256
PATCH = 3
PH = 86
C = 16
B = 32
NCORES = 8
M_TOTAL = B * PH * PH            # 236672 patch rows
NC_CORE = M_TOTAL // NCORES      # 29584 rows per core
TILE_N = 512
TPS = 4                          # tiles per super (strip-packed)
SUP = TILE_N * TPS               # 2048
NSUP = 15
NCP = NSUP * SUP                 # 30720 padded columns per core
NT = NCP // TILE_N               # 60
D_IN = PATCH * PATCH * C         # 144

TRACE = False
TRACE_KWARGS = {}
LAST_EXEC_NS = None
LAST_RESULT = None

# feature index maps into the raw (M, 144) patch matrix
_RESTIDX = [p * C + c for p in range(9) for c in range(1, C)]   # 135 rows
_CH0IDX = [p * C for p in range(9)]                              # 9 rows
_IDXA = np.tile(np.arange(9), 9)      # q=(u,v) -> v
_IDXB = np.repeat(np.arange(9), 9)    # q=(u,v) -> u


def _relabel_consts():
    TLT = np.zeros((81, 9), np.float32)   # S_u = sum_{v<u} eq[(u,v)]
    TINC = np.zeros((9, 9), np.float32)   # d_m = sum_{k<=m} fo_k
    for u in range(9):
        for v in range(9):
            if v < u:
                TLT[u * 9 + v, u] = 1.0
    for k in range(9):
        for m in range(9):
            if k <= m:
                TINC[k, m] = 1.0
    return TLT, TINC


_GRAPH = None


def _build_graph():
    global _GRAPH
    if _GRAPH is not None:
        return _GRAPH
    import concourse.bass as bass
    import concourse.bacc as bacc
    import concourse.mybir as mybir
    import concourse.tile as tile

    bf16 = mybir.dt.bfloat16
    f32 = mybir.dt.float32
    AF = mybir.ActivationFunctionType
    ALU = mybir.AluOpType

    nc = bacc.Bacc("TRN2")
    feat_e = nc.declare_dram_parameter("feat", [135, NCP], bf16, isOutput=False)
    rep_e = nc.declare_dram_parameter("rep", [162, NCP], bf16, isOutput=False)
    w1a_e = nc.declare_dram_parameter("w1a", [128, 256], bf16, isOutput=False)
    w1b_e = nc.declare_dram_parameter("w1b", [88, 256], bf16, isOutput=False)
    w2_e = nc.declare_dram_parameter("w2", [256, 256], bf16, isOutput=False)
    whx_e = nc.declare_dram_parameter("whx", [256, 8], bf16, isOutput=False)
    b1_e = nc.declare_dram_parameter("b1", [256, 1], f32, isOutput=False)
    b2_e = nc.declare_dram_parameter("b2", [256, 1], f32, isOutput=False)
    tlt_e = nc.declare_dram_parameter("tlt", [81, 9], bf16, isOutput=False)
    tinc_e = nc.declare_dram_parameter("tinc", [9, 9], bf16, isOutput=False)
    bfo_e = nc.declare_dram_parameter("bfo", [128, 1], f32, isOutput=False)
    out_e = nc.declare_dram_parameter("out", [128, NSUP * TILE_N], f32,
                                      isOutput=True)

    def scol(s):
        return bass.ts(s, SUP)

    def tcol(t):
        return bass.ts(t, TILE_N)

    with tile.TileContext(nc) as tc:
        with (
            tc.tile_pool(name="const", bufs=1) as cp,
            tc.tile_pool(name="sb", bufs=3) as sb,
            tc.tile_pool(name="ps", bufs=2, space=bass.MemorySpace.PSUM) as ps,
        ):
            def const_tile(src, shape, dtype, tag, eng=None):
                t = cp.tile(shape, dtype, tag=tag, name=tag)
                (eng or nc.sync).dma_start(t[:], src)
                return t

            gq = nc.gpsimd
            w1a0 = const_tile(w1a_e[:, 0:128], [128, 128], bf16, "w1a0", gq)
            w1a1 = const_tile(w1a_e[:, 128:256], [128, 128], bf16, "w1a1", gq)
            w1b0 = const_tile(w1b_e[:, 0:128], [88, 128], bf16, "w1b0", gq)
            w1b1 = const_tile(w1b_e[:, 128:256], [88, 128], bf16, "w1b1", gq)
            w2_00 = const_tile(w2_e[0:128, 0:128], [128, 128], bf16, "w2_00", gq)
            w2_01 = const_tile(w2_e[0:128, 128:256], [128, 128], bf16, "w2_01", gq)
            w2_10 = const_tile(w2_e[128:256, 0:128], [128, 128], bf16, "w2_10", gq)
            w2_11 = const_tile(w2_e[128:256, 128:256], [128, 128], bf16, "w2_11", gq)
            whx0 = const_tile(whx_e[0:128, :], [128, 8], bf16, "whx0", gq)
            whx1 = const_tile(whx_e[128:256, :], [128, 8], bf16, "whx1", gq)
            b1a = const_tile(b1_e[0:128, :], [128, 1], f32, "b1a", nc.scalar)
            b1b = const_tile(b1_e[128:256, :], [128, 1], f32, "b1b", nc.scalar)
            b2a = const_tile(b2_e[0:128, :], [128, 1], f32, "b2a", nc.scalar)
            b2b = const_tile(b2_e[128:256, :], [128, 1], f32, "b2b", nc.scalar)
            tlt = const_tile(tlt_e[:], [81, 9], bf16, "tlt", nc.sync)
            bfo = const_tile(bfo_e[:], [128, 1], f32, "bfo", nc.scalar)
            tincS = cp.tile([128, 9], bf16, tag="tincS", name="tincS")
            for t in range(TPS):
                nc.scalar.dma_start(tincS[32 * t:32 * t + 9, :], tinc_e[:])

            def in_rng(s):
                return 0 <= s < NSUP

            fa = {}; xb = {}; Br = {}; Ar = {}; EQ = {}; fo4 = {}; fd4 = {}
            FDr = {}; PR1 = {}; psSD = {}; psH = {}; otS = {}
            h1a = {}; h1b = {}; h2a = {}; h2b = {}
            ps1a = {}; ps1b = {}; ps2a = {}; ps2b = {}

            LAG = 3
            for k in range(NSUP + LAG):
                s = k
                if in_rng(s):  # LD: feature + pair-replication loads
                    fa[s] = sb.tile([128, SUP], bf16, tag="fa", bufs=4,
                                    name=f"fa_{s}")
                    nc.sync.dma_start(fa[s][:], feat_e[0:128, scol(s)])
                    xb[s] = sb.tile([88, SUP], bf16, tag="xb", bufs=3,
                                    name=f"xb_{s}")
                    nc.sync.dma_start(xb[s][81:88, :], feat_e[128:135, scol(s)])
                    Ar[s] = sb.tile([81, SUP], bf16, tag="Ar", bufs=3,
                                    name=f"Ar_{s}")
                    Br[s] = sb.tile([81, SUP], bf16, tag="Br", bufs=4,
                                    name=f"Br_{s}")
                    nc.sync.dma_start(Ar[s][:], rep_e[0:81, scol(s)])
                    nc.sync.dma_start(Br[s][:], rep_e[81:162, scol(s)])

                s = k - 1
                if in_rng(s):  # relabel chain for super s
                    EQ[s] = sb.tile([81, SUP], bf16, tag="EQ", bufs=3,
                                    name=f"EQ_{s}")
                    nc.vector.scalar_tensor_tensor(
                        EQ[s][:], Ar[s][:], 0.0, Br[s][:],
                        op0=ALU.bypass, op1=ALU.is_equal)
                    del Ar[s]
                    # S: 4 col-tiled concurrent matmuls into one bank
                    psSD[s] = ps.tile([128, TILE_N], f32, tag="sd", bufs=1,
                                      name=f"psSD_{s}")
                    for t in range(TPS):
                        nc.tensor.matmul(psSD[s][32 * t:32 * t + 9, :], tlt[:],
                                         EQ[s][:, tcol(t)], start=True,
                                         stop=True, tile_position=(0, 32 * t))
                    fo4[s] = sb.tile([128, TILE_N], bf16, tag="fo4", bufs=2,
                                     name=f"fo4_{s}")
                    nc.scalar.activation(fo4[s][:], psSD[s][:], AF.Relu,
                                         bias=bfo[:], scale=-1.0)
                    # D: 4 diag-tiled concurrent matmuls, reusing the bank
                    for t in range(TPS):
                        nc.tensor.matmul(psSD[s][32 * t:32 * t + 9, :],
                                         tincS[32 * t:32 * t + 9, :],
                                         fo4[s][32 * t:32 * t + 9, :],
                                         start=True, stop=True,
                                         tile_position=(32 * t, 32 * t))
                    fd4[s] = sb.tile([128, TILE_N], bf16, tag="fd4", bufs=2,
                                     name=f"fd4_{s}")
                    nc.vector.scalar_tensor_tensor(
                        fd4[s][:], fo4[s][:], 0.0, psSD[s][:],
                        op0=ALU.bypass, op1=ALU.mult)
                    del psSD[s]; del fo4[s]
                    # FDrep: strips -> rows 0:9, then log-doubling to 81
                    FDr[s] = sb.tile([81, SUP], bf16, tag="FDr", bufs=2,
                                     name=f"FDr_{s}")
                    for t in range(TPS):
                        gq.dma_start(FDr[s][0:9, tcol(t)],
                                     fd4[s][32 * t:32 * t + 9, :])
                    gq.dma_start(FDr[s][9:18, :], FDr[s][0:9, :])
                    gq.dma_start(FDr[s][18:36, :], FDr[s][0:18, :])
                    gq.dma_start(FDr[s][36:72, :], FDr[s][0:36, :])
                    gq.dma_start(FDr[s][72:81, :], FDr[s][0:9, :])
                    del fd4[s]

                s = k - 2
                if in_rng(s):  # PR' -> xb rows 0:81
                    PR1[s] = sb.tile([81, SUP], bf16, tag="PR1", bufs=2,
                                     name=f"PR1_{s}")
                    nc.vector.scalar_tensor_tensor(
                        PR1[s][:], EQ[s][:], 0.0, FDr[s][:],
                        op0=ALU.bypass, op1=ALU.mult)
                    del EQ[s]; del FDr[s]
                    nc.vector.scalar_tensor_tensor(
                        xb[s][0:81, :], Br[s][:], 0.0, PR1[s][:],
                        op0=ALU.not_equal, op1=ALU.mult)
                    del Br[s]; del PR1[s]

                s = k - LAG
                if in_rng(s):  # MLP for the 4 tiles of super s
                    psH[s] = ps.tile([128, TILE_N], f32, tag="psH", bufs=1,
                                     name=f"psH_{s}")
                    for tp in range(TPS // 2):  # weight-grouped tile pairs
                        tt2 = (2 * tp, 2 * tp + 1)
                        for t in tt2:
                            ps1a[t] = ps.tile([128, TILE_N], f32, tag="ps1",
                                              bufs=4, name=f"ps1a_{s}_{t}")
                            ps1b[t] = ps.tile([128, TILE_N], f32, tag="ps1",
                                              bufs=4, name=f"ps1b_{s}_{t}")
                        for t in tt2:
                            nc.tensor.matmul(ps1a[t][:], w1a0[:],
                                             fa[s][:, tcol(t)],
                                             start=True, stop=False)
                        for t in tt2:
                            nc.tensor.matmul(ps1a[t][:], w1b0[:],
                                             xb[s][:, tcol(t)],
                                             start=False, stop=True)
                        for t in tt2:
                            nc.tensor.matmul(ps1b[t][:], w1a1[:],
                                             fa[s][:, tcol(t)],
                                             start=True, stop=False)
                        for t in tt2:
                            nc.tensor.matmul(ps1b[t][:], w1b1[:],
                                             xb[s][:, tcol(t)],
                                             start=False, stop=True)
                        for t in tt2:
                            h1a[t] = sb.tile([128, TILE_N], bf16, tag="h1a",
                                             bufs=4, name=f"h1a_{s}_{t}")
                            h1b[t] = sb.tile([128, TILE_N], bf16, tag="h1b",
                                             bufs=4, name=f"h1b_{s}_{t}")
                            nc.scalar.activation(h1a[t][:], ps1a[t][:],
                                                 AF.Relu, bias=b1a[:])
                            nc.scalar.activation(h1b[t][:], ps1b[t][:],
                                                 AF.Relu, bias=b1b[:])
                            del ps1a[t]; del ps1b[t]
                        for t in tt2:
                            ps2a[t] = ps.tile([128, TILE_N], f32, tag="ps2",
                                              bufs=2, name=f"ps2a_{s}_{t}")
                            nc.tensor.matmul(ps2a[t][:], w2_00[:], h1a[t][:],
                                             start=True, stop=False)
                            nc.tensor.matmul(ps2a[t][:], w2_10[:], h1b[t][:],
                                             start=False, stop=True)
                            ps2b[t] = ps.tile([128, TILE_N], f32, tag="ps2",
                                              bufs=2, name=f"ps2b_{s}_{t}")
                            nc.tensor.matmul(ps2b[t][:], w2_01[:], h1a[t][:],
                                             start=True, stop=False)
                            nc.tensor.matmul(ps2b[t][:], w2_11[:], h1b[t][:],
                                             start=False, stop=True)
                            h2a[t] = sb.tile([128, TILE_N], bf16, tag="h2a",
                                             bufs=3, name=f"h2a_{s}_{t}")
                            h2b[t] = sb.tile([128, TILE_N], bf16, tag="h2b",
                                             bufs=3, name=f"h2b_{s}_{t}")
                            if t == 0:
                                nc.scalar.activation(h2a[t][:], ps2a[t][:],
                                                     AF.Relu, bias=b2a[:])
                            else:
                                nc.vector.tensor_scalar(
                                    h2a[t][:], ps2a[t][:], b2a[:], 0.0,
                                    op0=ALU.add, op1=ALU.max)
                            nc.vector.tensor_scalar(
                                h2b[t][:], ps2b[t][:], b2b[:], 0.0,
                                op0=ALU.add, op1=ALU.max)
                            del ps2a[t]; del ps2b[t]
                            del h1a[t]; del h1b[t]
                    del fa[s]; del xb[s]
                    # L3 + heads: one open accumulation group per bank at a
                    # time -- start/stop interleaved per 32-row strip
                    for t in range(TPS):
                        nc.tensor.matmul(psH[s][32 * t:32 * t + 8, :], whx0[:],
                                         h2a[t][:], start=True, stop=False,
                                         tile_position=(0, 32 * t))
                        nc.tensor.matmul(psH[s][32 * t:32 * t + 8, :], whx1[:],
                                         h2b[t][:], start=False, stop=True,
                                         tile_position=(0, 32 * t))
                    for t in range(TPS):
                        del h2a[t]; del h2b[t]
                    otS[s] = sb.tile([128, TILE_N], f32, tag="otS", bufs=2,
                                     name=f"otS_{s}")
                    nc.vector.tensor_copy(otS[s][:], psH[s][:])
                    del psH[s]
                    nc.sync.dma_start(out_e[:, bass.ts(s, TILE_N)], otS[s][:])
                    del otS[s]

    nc.finalize()
    _GRAPH = nc
    return nc


def _extract_features(x):
    """numpy port of the reference's offset decode + patch extraction."""
    x = np.array(x, dtype=np.float32, copy=True)
    code = x[0, 0, 0, 0]
    it = np.float32(np.mod(code, np.float32(100.0)))
    x[0, 0, 0, 0] = np.float32((code - it) / np.float32(100.0))
    it_i = np.int32(it)
    off_h = int(it_i % 3)
    off_w = int((it_i // 3) % 3)
    xp = np.zeros((B, H + 4, W + 4, C), np.float32)
    xp[:, 2:2 + H, 2:2 + W, :] = x
    xp = xp[:, 2 - off_h:2 - off_h + H + 2, 2 - off_w:2 - off_w + W + 2, :]
    patches = xp.reshape(B, PH, PATCH, PH, PATCH, C)
    patches = patches.transpose(0, 1, 3, 2, 4, 5).reshape(M_TOTAL, PATCH * PATCH, C)
    return patches.reshape(M_TOTAL, D_IN)


_BFO = np.ones((128, 1), np.float32)


def kernel(x, W1, b1, W2, b2, W3, b3, Wm, bm, Wl, bl):
    global LAST_EXEC_NS, LAST_RESULT
    from concourse.bass_utils import run_bass_kernel_spmd
    import concourse.mybir as mybir

    bf16 = mybir.dt.np(mybir.dt.bfloat16)
    feat = _extract_features(np.asarray(x))

    TLT, TINC = _relabel_consts()
    W1 = np.asarray(W1, np.float32)
    W3 = np.asarray(W3, np.float32)
    b3 = np.asarray(b3, np.float32)
    Wm = np.asarray(Wm, np.float32)
    Wl = np.asarray(Wl, np.float32)
    whx = W3 @ np.concatenate([Wm, Wl], axis=1)          # [256, 8]
    bias8 = np.concatenate([b3 @ Wm + np.asarray(bm, np.float32),
                            b3 @ Wl + np.asarray(bl, np.float32)])  # [8]
    W1rest = W1[_RESTIDX, :]                              # [135, 256]
    W1ch0 = W1[_CH0IDX, :]                                # [9, 256]
    w1a = W1rest[0:128]
    w1b = np.concatenate([W1ch0[_IDXB], W1rest[128:135]], axis=0)  # [88, 256]
    common = dict(
        w1a=w1a.astype(bf16), w1b=np.ascontiguousarray(w1b).astype(bf16),
        w2=np.asarray(W2, np.float32).astype(bf16),
        whx=whx.astype(bf16),
        b1=np.asarray(b1, np.float32).reshape(256, 1),
        b2=np.asarray(b2, np.float32).reshape(256, 1),
        tlt=TLT.astype(bf16), tinc=TINC.astype(bf16),
        bfo=_BFO,
    )
    restT = feat[:, _RESTIDX].astype(bf16)                # [M, 135]
    ch0 = feat[:, _CH0IDX].astype(bf16)                   # [M, 9]
    in_maps = []
    for c in range(NCORES):
        lo, hi = c * NC_CORE, (c + 1) * NC_CORE
        ft = np.zeros((135, NCP), bf16)
        ft[:, :NC_CORE] = restT[lo:hi].T
        rp = np.zeros((162, NCP), bf16)
        rp[0:81, :NC_CORE] = ch0[lo:hi][:, _IDXA].T
        rp[81:162, :NC_CORE] = ch0[lo:hi][:, _IDXB].T
        in_maps.append(dict(feat=np.ascontiguousarray(ft),
                            rep=np.ascontiguousarray(rp), **common))

    nc = _build_graph()
    res = run_bass_kernel_spmd(
        nc, in_maps, list(range(NCORES)), trace=TRACE, trace_kwargs=TRACE_KWARGS)
    LAST_EXEC_NS = res.exec_time_ns
    LAST_RESULT = res
    means, logs = [], []
    for c in range(NCORES):
        raw = res.results[c]["out"]                       # [128, NSUP*512]
        # row 32t+h, col 512s+cc  ->  head h of tile (4s+t)
        o = raw.reshape(4, 32, NSUP, TILE_N)[:, 0:8]      # [t, h, s, cc]
        o = o.transpose(1, 2, 0, 3).reshape(8, NCP)[:, :NC_CORE]
        o = o + bias8[:, None]
        means.append(o[0:4].T.reshape(B // NCORES, PH * PH * 4))
        logs.append(o[4:8].T.reshape(B // NCORES, PH * PH * 4))
    mean = np.concatenate(means, axis=0)
    log_std = np.concatenate(logs, axis=0)
    return mean, log_std


# revision 19
# speedup vs baseline: 1.5308x; 1.0295x over previous
"""Trainium2 Bass kernel for the Actor net (patch relabel + MLP), 8-core SPMD.

Strategy: data-parallel over the B*7396 patch-row axis. Host extracts the
3x3 non-overlapping patches (offset decoded from x[0,0,0,0]) into a
feature-major tensor, sharded by rows across 8 cores. Single fused
software-pipelined loop (keeps the PE HAM clock-gate warm at 2.4 GHz).

Relabel (per 2048-col super, 4 tiles strip-packed at 32-row offsets):
  - Host ships pair-space replications of the 9 ch0 values: A[q=(u,v)]=a_v,
    B[q]=a_u (81 rows each). Device: EQ = (A==B) on DVE.
  - S (dup-count before u) via one 4-way col-tiled matmul group; fo=relu(1-S)
    on Act; d (cumulative first-occurrence count) via 4-way diag-tiled
    matmuls; fd = fo*d on DVE.
  - fd strips are replicated to 81 pair rows by a log-doubling DMA chain.
  - PR' = (B!=0) * EQ * FDrep; the uni contraction (TALL) is folded into
    the first MLP layer: W1B rows 0:81 replicate W1's ch0 rows, so
    L1 = W1A^T @ rest128 + W1B^T @ [PR'; rest7]  (K=128 + K=88).
MLP: L2 256x256, L3+heads folded into one [256,8] matrix (host folds W3 and
the two heads; output biases added on the host).
"""
import sys

sys.path.insert(0, "/opt/trn_rl_repo")

import numpy as np

H = W = 256
PATCH = 3
PH = 86
C = 16
B = 32
NCORES = 8
M_TOTAL = B * PH * PH            # 236672 patch rows
NC_CORE = M_TOTAL // NCORES      # 29584 rows per core
TILE_N = 512
TPS = 4                          # tiles per super (strip-packed)
SUP = TILE_N * TPS               # 2048
NSUP = 15
NCP = NSUP * SUP                 # 30720 padded columns per core
NT = NCP // TILE_N               # 60
D_IN = PATCH * PATCH * C         # 144

TRACE = False
TRACE_KWARGS = {}
LAST_EXEC_NS = None
LAST_RESULT = None

# feature index maps into the raw (M, 144) patch matrix
_RESTIDX = [p * C + c for p in range(9) for c in range(1, C)]   # 135 rows
_CH0IDX = [p * C for p in range(9)]                              # 9 rows
_IDXA = np.tile(np.arange(9), 9)      # q=(u,v) -> v
_IDXB = np.repeat(np.arange(9), 9)    # q=(u,v) -> u


def _relabel_consts():
    TLT = np.zeros((81, 9), np.float32)   # S_u = sum_{v<u} eq[(u,v)]
    TINC = np.zeros((9, 9), np.float32)   # d_m = sum_{k<=m} fo_k
    for u in range(9):
        for v in range(9):
            if v < u:
                TLT[u * 9 + v, u] = 1.0
    for k in range(9):
        for m in range(9):
            if k <= m:
                TINC[k, m] = 1.0
    return TLT, TINC


_GRAPH = None


def _build_graph():
    global _GRAPH
    if _GRAPH is not None:
        return _GRAPH
    import concourse.bass as bass
    import concourse.bacc as bacc
    import concourse.mybir as mybir
    import concourse.tile as tile

    bf16 = mybir.dt.bfloat16
    f32 = mybir.dt.float32
    AF = mybir.ActivationFunctionType
    ALU = mybir.AluOpType

    nc = bacc.Bacc("TRN2")
    feat_e = nc.declare_dram_parameter("feat", [135, NCP], bf16, isOutput=False)
    rep_e = nc.declare_dram_parameter("rep", [162, NCP], bf16, isOutput=False)
    w1a_e = nc.declare_dram_parameter("w1a", [128, 256], bf16, isOutput=False)
    w1b_e = nc.declare_dram_parameter("w1b", [88, 256], bf16, isOutput=False)
    w2_e = nc.declare_dram_parameter("w2", [256, 256], bf16, isOutput=False)
    whx_e = nc.declare_dram_parameter("whx", [256, 8], bf16, isOutput=False)
    b1_e = nc.declare_dram_parameter("b1", [256, 1], f32, isOutput=False)
    b2_e = nc.declare_dram_parameter("b2", [256, 1], f32, isOutput=False)
    tlt_e = nc.declare_dram_parameter("tlt", [81, 9], bf16, isOutput=False)
    tinc_e = nc.declare_dram_parameter("tinc", [9, 9], bf16, isOutput=False)
    bfo_e = nc.declare_dram_parameter("bfo", [128, 1], f32, isOutput=False)
    out_e = nc.declare_dram_parameter("out", [128, NSUP * TILE_N], f32,
                                      isOutput=True)

    def scol(s):
        return bass.ts(s, SUP)

    def tcol(t):
        return bass.ts(t, TILE_N)

    with tile.TileContext(nc) as tc:
        with (
            tc.tile_pool(name="const", bufs=1) as cp,
            tc.tile_pool(name="sb", bufs=3) as sb,
            tc.tile_pool(name="ps", bufs=2, space=bass.MemorySpace.PSUM) as ps,
        ):
            def const_tile(src, shape, dtype, tag, eng=None):
                t = cp.tile(shape, dtype, tag=tag, name=tag)
                (eng or nc.sync).dma_start(t[:], src)
                return t

            gq = nc.gpsimd
            w1a0 = const_tile(w1a_e[:, 0:128], [128, 128], bf16, "w1a0", gq)
            w1a1 = const_tile(w1a_e[:, 128:256], [128, 128], bf16, "w1a1", gq)
            w1b0 = const_tile(w1b_e[:, 0:128], [88, 128], bf16, "w1b0", gq)
            w1b1 = const_tile(w1b_e[:, 128:256], [88, 128], bf16, "w1b1", gq)
            w2_00 = const_tile(w2_e[0:128, 0:128], [128, 128], bf16, "w2_00", gq)
            w2_01 = const_tile(w2_e[0:128, 128:256], [128, 128], bf16, "w2_01", gq)
            w2_10 = const_tile(w2_e[128:256, 0:128], [128, 128], bf16, "w2_10", gq)
            w2_11 = const_tile(w2_e[128:256, 128:256], [128, 128], bf16, "w2_11", gq)
            whx0 = const_tile(whx_e[0:128, :], [128, 8], bf16, "whx0", gq)
            whx1 = const_tile(whx_e[128:256, :], [128, 8], bf16, "whx1", gq)
            b1a = const_tile(b1_e[0:128, :], [128, 1], f32, "b1a", nc.scalar)
            b1b = const_tile(b1_e[128:256, :], [128, 1], f32, "b1b", nc.scalar)
            b2a = const_tile(b2_e[0:128, :], [128, 1], f32, "b2a", nc.scalar)
            b2b = const_tile(b2_e[128:256, :], [128, 1], f32, "b2b", nc.scalar)
            tlt = const_tile(tlt_e[:], [81, 9], bf16, "tlt", nc.sync)
            bfo = const_tile(bfo_e[:], [128, 1], f32, "bfo", nc.scalar)
            tincS = cp.tile([128, 9], bf16, tag="tincS", name="tincS")
            for t in range(TPS):
                nc.scalar.dma_start(tincS[32 * t:32 * t + 9, :], tinc_e[:])

            def in_rng(s):
                return 0 <= s < NSUP

            fa = {}; xb = {}; Br = {}; Ar = {}; EQ = {}; fo4 = {}; fd4 = {}
            FDr = {}; PR1 = {}; psSD = {}; psH = {}; otS = {}
            h1a = {}; h1b = {}; h2a = {}; h2b = {}
            ps1a = {}; ps1b = {}; ps2a = {}; ps2b = {}

            LAG = 3
            for k in range(NSUP + LAG):
                s = k
                if in_rng(s):  # LD: feature + pair-replication loads
                    fa[s] = sb.tile([128, SUP], bf16, tag="fa", bufs=4,
                                    name=f"fa_{s}")
                    nc.sync.dma_start(fa[s][:], feat_e[0:128, scol(s)])
                    xb[s] = sb.tile([88, SUP], bf16, tag="xb", bufs=3,
                                    name=f"xb_{s}")
                    nc.sync.dma_start(xb[s][81:88, :], feat_e[128:135, scol(s)])
                    Ar[s] = sb.tile([81, SUP], bf16, tag="Ar", bufs=3,
                                    name=f"Ar_{s}")
                    Br[s] = sb.tile([81, SUP], bf16, tag="Br", bufs=4,
                                    name=f"Br_{s}")
                    nc.sync.dma_start(Ar[s][:], rep_e[0:81, scol(s)])
                    nc.sync.dma_start(Br[s][:], rep_e[81:162, scol(s)])

                s = k - 1
                if in_rng(s):  # relabel chain for super s
                    EQ[s] = sb.tile([81, SUP], bf16, tag="EQ", bufs=3,
                                    name=f"EQ_{s}")
                    nc.vector.scalar_tensor_tensor(
                        EQ[s][:], Ar[s][:], 0.0, Br[s][:],
                        op0=ALU.bypass, op1=ALU.is_equal)
                    del Ar[s]
                    # S: 4 col-tiled concurrent matmuls into one bank
                    psSD[s] = ps.tile([128, TILE_N], f32, tag="sd", bufs=1,
                                      name=f"psSD_{s}")
                    for t in range(TPS):
                        nc.tensor.matmul(psSD[s][32 * t:32 * t + 9, :], tlt[:],
                                         EQ[s][:, tcol(t)], start=True,
                                         stop=True, tile_position=(0, 32 * t))
                    fo4[s] = sb.tile([128, TILE_N], bf16, tag="fo4", bufs=2,
                                     name=f"fo4_{s}")
                    nc.scalar.activation(fo4[s][:], psSD[s][:], AF.Relu,
                                         bias=bfo[:], scale=-1.0)
                    # D: 4 diag-tiled concurrent matmuls, reusing the bank
                    for t in range(TPS):
                        nc.tensor.matmul(psSD[s][32 * t:32 * t + 9, :],
                                         tincS[32 * t:32 * t + 9, :],
                                         fo4[s][32 * t:32 * t + 9, :],
                                         start=True, stop=True,
                                         tile_position=(32 * t, 32 * t))
                    fd4[s] = sb.tile([128, TILE_N], bf16, tag="fd4", bufs=2,
                                     name=f"fd4_{s}")
                    nc.vector.scalar_tensor_tensor(
                        fd4[s][:], fo4[s][:], 0.0, psSD[s][:],
                        op0=ALU.bypass, op1=ALU.mult)
                    del psSD[s]; del fo4[s]
                    # FDrep: strips -> rows 0:9, then log-doubling to 81
                    FDr[s] = sb.tile([81, SUP], bf16, tag="FDr", bufs=2,
                                     name=f"FDr_{s}")
                    for t in range(TPS):
                        gq.dma_start(FDr[s][0:9, tcol(t)],
                                     fd4[s][32 * t:32 * t + 9, :])
                    gq.dma_start(FDr[s][9:18, :], FDr[s][0:9, :])
                    gq.dma_start(FDr[s][18:36, :], FDr[s][0:18, :])
                    gq.dma_start(FDr[s][36:72, :], FDr[s][0:36, :])
                    gq.dma_start(FDr[s][72:81, :], FDr[s][0:9, :])
                    del fd4[s]

                s = k - 2
                if in_rng(s):  # PR' -> xb rows 0:81
                    PR1[s] = sb.tile([81, SUP], bf16, tag="PR1", bufs=2,
                                     name=f"PR1_{s}")
                    nc.vector.scalar_tensor_tensor(
                        PR1[s][:], EQ[s][:], 0.0, FDr[s][:],
                        op0=ALU.bypass, op1=ALU.mult)
                    del EQ[s]; del FDr[s]
                    nc.vector.scalar_tensor_tensor(
                        xb[s][0:81, :], Br[s][:], 0.0, PR1[s][:],
                        op0=ALU.not_equal, op1=ALU.mult)
                    del Br[s]; del PR1[s]

                s = k - LAG
                if in_rng(s):  # MLP for the 4 tiles of super s
                    psH[s] = ps.tile([128, TILE_N], f32, tag="psH", bufs=1,
                                     name=f"psH_{s}")
                    for tp in range(TPS // 2):  # weight-grouped tile pairs
                        tt2 = (2 * tp, 2 * tp + 1)
                        for t in tt2:
                            ps1a[t] = ps.tile([128, TILE_N], f32, tag="ps1",
                                              bufs=4, name=f"ps1a_{s}_{t}")
                            ps1b[t] = ps.tile([128, TILE_N], f32, tag="ps1",
                                              bufs=4, name=f"ps1b_{s}_{t}")
                        for t in tt2:
                            nc.tensor.matmul(ps1a[t][:], w1a0[:],
                                             fa[s][:, tcol(t)],
                                             start=True, stop=False)
                        for t in tt2:
                            nc.tensor.matmul(ps1a[t][:], w1b0[:],
                                             xb[s][:, tcol(t)],
                                             start=False, stop=True)
                        for t in tt2:
                            nc.tensor.matmul(ps1b[t][:], w1a1[:],
                                             fa[s][:, tcol(t)],
                                             start=True, stop=False)
                        for t in tt2:
                            nc.tensor.matmul(ps1b[t][:], w1b1[:],
                                             xb[s][:, tcol(t)],
                                             start=False, stop=True)
                        for t in tt2:
                            h1a[t] = sb.tile([128, TILE_N], bf16, tag="h1a",
                                             bufs=4, name=f"h1a_{s}_{t}")
                            h1b[t] = sb.tile([128, TILE_N], bf16, tag="h1b",
                                             bufs=4, name=f"h1b_{s}_{t}")
                            nc.scalar.activation(h1a[t][:], ps1a[t][:],
                                                 AF.Relu, bias=b1a[:])
                            nc.scalar.activation(h1b[t][:], ps1b[t][:],
                                                 AF.Relu, bias=b1b[:])
                            del ps1a[t]; del ps1b[t]
                        for t in tt2:
                            ps2a[t] = ps.tile([128, TILE_N], f32, tag="ps2",
                                              bufs=2, name=f"ps2a_{s}_{t}")
                            nc.tensor.matmul(ps2a[t][:], w2_00[:], h1a[t][:],
                                             start=True, stop=False)
                            nc.tensor.matmul(ps2a[t][:], w2_10[:], h1b[t][:],
                                             start=False, stop=True)
                            ps2b[t] = ps.tile([128, TILE_N], f32, tag="ps2",
                                              bufs=2, name=f"ps2b_{s}_{t}")
                            nc.tensor.matmul(ps2b[t][:], w2_01[:], h1a[t][:],
                                             start=True, stop=False)
                            nc.tensor.matmul(ps2b[t][:], w2_11[:], h1b[t][:],
                                             start=False, stop=True)
                            h2a[t] = sb.tile([128, TILE_N], bf16, tag="h2a",
                                             bufs=3, name=f"h2a_{s}_{t}")
                            h2b[t] = sb.tile([128, TILE_N], bf16, tag="h2b",
                                             bufs=3, name=f"h2b_{s}_{t}")
                            if t <= 2:
                                nc.scalar.activation(h2a[t][:], ps2a[t][:],
                                                     AF.Relu, bias=b2a[:])
                            else:
                                nc.vector.tensor_scalar(
                                    h2a[t][:], ps2a[t][:], b2a[:], 0.0,
                                    op0=ALU.add, op1=ALU.max)
                            nc.vector.tensor_scalar(
                                h2b[t][:], ps2b[t][:], b2b[:], 0.0,
                                op0=ALU.add, op1=ALU.max)
                            del ps2a[t]; del ps2b[t]
                            del h1a[t]; del h1b[t]
                    del fa[s]; del xb[s]
                    # L3 + heads: one open accumulation group per bank at a
                    # time -- start/stop interleaved per 32-row strip
                    for t in range(TPS):
                        nc.tensor.matmul(psH[s][32 * t:32 * t + 8, :], whx0[:],
                                         h2a[t][:], start=True, stop=False,
                                         tile_position=(0, 32 * t))
                        nc.tensor.matmul(psH[s][32 * t:32 * t + 8, :], whx1[:],
                                         h2b[t][:], start=False, stop=True,
                                         tile_position=(0, 32 * t))
                    for t in range(TPS):
                        del h2a[t]; del h2b[t]
                    otS[s] = sb.tile([128, TILE_N], f32, tag="otS", bufs=2,
                                     name=f"otS_{s}")
                    nc.scalar.activation(otS[s][:], psH[s][:], AF.Copy)
                    del psH[s]
                    nc.sync.dma_start(out_e[:, bass.ts(s, TILE_N)], otS[s][:])
                    del otS[s]

    nc.finalize()
    _GRAPH = nc
    return nc


def _extract_features(x):
    """numpy port of the reference's offset decode + patch extraction."""
    x = np.array(x, dtype=np.float32, copy=True)
    code = x[0, 0, 0, 0]
    it = np.float32(np.mod(code, np.float32(100.0)))
    x[0, 0, 0, 0] = np.float32((code - it) / np.float32(100.0))
    it_i = np.int32(it)
    off_h = int(it_i % 3)
    off_w = int((it_i // 3) % 3)
    xp = np.zeros((B, H + 4, W + 4, C), np.float32)
    xp[:, 2:2 + H, 2:2 + W, :] = x
    xp = xp[:, 2 - off_h:2 - off_h + H + 2, 2 - off_w:2 - off_w + W + 2, :]
    patches = xp.reshape(B, PH, PATCH, PH, PATCH, C)
    patches = patches.transpose(0, 1, 3, 2, 4, 5).reshape(M_TOTAL, PATCH * PATCH, C)
    return patches.reshape(M_TOTAL, D_IN)


_BFO = np.ones((128, 1), np.float32)


def kernel(x, W1, b1, W2, b2, W3, b3, Wm, bm, Wl, bl):
    global LAST_EXEC_NS, LAST_RESULT
    from concourse.bass_utils import run_bass_kernel_spmd
    import concourse.mybir as mybir

    bf16 = mybir.dt.np(mybir.dt.bfloat16)
    feat = _extract_features(np.asarray(x))

    TLT, TINC = _relabel_consts()
    W1 = np.asarray(W1, np.float32)
    W3 = np.asarray(W3, np.float32)
    b3 = np.asarray(b3, np.float32)
    Wm = np.asarray(Wm, np.float32)
    Wl = np.asarray(Wl, np.float32)
    whx = W3 @ np.concatenate([Wm, Wl], axis=1)          # [256, 8]
    bias8 = np.concatenate([b3 @ Wm + np.asarray(bm, np.float32),
                            b3 @ Wl + np.asarray(bl, np.float32)])  # [8]
    W1rest = W1[_RESTIDX, :]                              # [135, 256]
    W1ch0 = W1[_CH0IDX, :]                                # [9, 256]
    w1a = W1rest[0:128]
    w1b = np.concatenate([W1ch0[_IDXB], W1rest[128:135]], axis=0)  # [88, 256]
    common = dict(
        w1a=w1a.astype(bf16), w1b=np.ascontiguousarray(w1b).astype(bf16),
        w2=np.asarray(W2, np.float32).astype(bf16),
        whx=whx.astype(bf16),
        b1=np.asarray(b1, np.float32).reshape(256, 1),
        b2=np.asarray(b2, np.float32).reshape(256, 1),
        tlt=TLT.astype(bf16), tinc=TINC.astype(bf16),
        bfo=_BFO,
    )
    restT = feat[:, _RESTIDX].astype(bf16)                # [M, 135]
    ch0 = feat[:, _CH0IDX].astype(bf16)                   # [M, 9]
    in_maps = []
    for c in range(NCORES):
        lo, hi = c * NC_CORE, (c + 1) * NC_CORE
        ft = np.zeros((135, NCP), bf16)
        ft[:, :NC_CORE] = restT[lo:hi].T
        rp = np.zeros((162, NCP), bf16)
        rp[0:81, :NC_CORE] = ch0[lo:hi][:, _IDXA].T
        rp[81:162, :NC_CORE] = ch0[lo:hi][:, _IDXB].T
        in_maps.append(dict(feat=np.ascontiguousarray(ft),
                            rep=np.ascontiguousarray(rp), **common))

    nc = _build_graph()
    res = run_bass_kernel_spmd(
        nc, in_maps, list(range(NCORES)), trace=TRACE, trace_kwargs=TRACE_KWARGS)
    LAST_EXEC_NS = res.exec_time_ns
    LAST_RESULT = res
    means, logs = [], []
    for c in range(NCORES):
        raw = res.results[c]["out"]                       # [128, NSUP*512]
        # row 32t+h, col 512s+cc  ->  head h of tile (4s+t)
        o = raw.reshape(4, 32, NSUP, TILE_N)[:, 0:8]      # [t, h, s, cc]
        o = o.transpose(1, 2, 0, 3).reshape(8, NCP)[:, :NC_CORE]
        o = o + bias8[:, None]
        means.append(o[0:4].T.reshape(B // NCORES, PH * PH * 4))
        logs.append(o[4:8].T.reshape(B // NCORES, PH * PH * 4))
    mean = np.concatenate(means, axis=0)
    log_std = np.concatenate(logs, axis=0)
    return mean, log_std
